# revision 1
# baseline (speedup 1.0000x reference)
"""DiffAttention Trainium2 kernel (8 NeuronCores, head-parallel).

Sharding: 16 heads / 8 cores = 2 heads per core (tensor-parallel style).
Each core: column-sharded QKV projections for its 2 heads, full attention
over T=2048, row-sharded output projection producing a partial [T, D]
output; the host sums the 8 partials.

Per-head math (reference):
  a1 = softmax(s1), a2 = softmax(s2), attn = a1 - lam*a2
  attn /= max(sum|attn|, 1e-6); out = attn @ v
Rewritten (1/l1 cancels; lam in (0,1) so the 1e-6 clamp is inactive):
  u  = p1 - c*p2           with p_i = exp(SCALE*s_i) (masked->0), c = lam*l1/l2
  attn = u / sum_k|u|,     sum_k|u| = (1+lam)*l1 - 2*sum_k min(p1, c*p2)
All big tensors live transposed ([k, q]) so per-q reductions are PE
ones-matmuls and attn@V consumes attn directly as lhsT.
"""

import os
from contextlib import ExitStack

import ml_dtypes
import numpy as np

import concourse.bass as bass
import concourse.tile as tile
from concourse import bacc, mybir
from concourse.bass import ts
from concourse.bass_utils import run_bass_kernel_spmd

F32 = mybir.dt.float32
F32R = mybir.dt.float32r
BF16 = mybir.dt.bfloat16
AF = mybir.ActivationFunctionType
ALU = mybir.AluOpType
BF16NP = ml_dtypes.bfloat16

D_MODEL = 1024
N_HEADS = 16
HEAD_DIM = 64
SCALE = HEAD_DIM**-0.5
N_CORES = 8
HPC = N_HEADS // N_CORES  # heads per core = 2
DC = HPC * HEAD_DIM  # projection dims per core = 128
NEG = -1.0e30


def _bcast_ap(row_ap: bass.AP, nparts: int) -> bass.AP:
    """Partition-broadcast view of a single-partition AP (for DMA reads)."""
    return bass.AP(
        tensor=row_ap.tensor,
        offset=row_ap.offset,
        ap=[[0, nparts]] + [list(d) for d in row_ap.ap[1:]],
    )


def _build(T: int):
    NQ = 512
    NQT = T // NQ
    NKB = T // 128
    nc = bacc.Bacc(
        "TRN2", target_bir_lowering=False, debug=False, num_devices=N_CORES
    )

    xT = nc.dram_tensor("xT", [D_MODEL, T], F32R, kind="ExternalInput").ap()
    xTb = nc.dram_tensor("xTb", [D_MODEL, T], BF16, kind="ExternalInput").ap()
    wq = nc.dram_tensor("wq", [D_MODEL, DC], F32R, kind="ExternalInput").ap()
    wk1 = nc.dram_tensor("wk1", [D_MODEL, DC], F32R, kind="ExternalInput").ap()
    wk2 = nc.dram_tensor("wk2", [D_MODEL, DC], F32R, kind="ExternalInput").ap()
    wv = nc.dram_tensor("wv", [D_MODEL, DC], BF16, kind="ExternalInput").ap()
    wo = nc.dram_tensor("wo", [DC, D_MODEL], F32R, kind="ExternalInput").ap()
    lamones = nc.dram_tensor("lamones", [128, HPC], BF16, kind="ExternalInput").ap()
    ones1 = nc.dram_tensor("ones1", [128, 1], BF16, kind="ExternalInput").ap()
    twos = nc.dram_tensor("twos", [128, 1], BF16, kind="ExternalInput").ap()
    maskb = nc.dram_tensor("maskb", [128, 128], BF16, kind="ExternalInput").ap()
    idneg = nc.dram_tensor("idneg", [128, 128], BF16, kind="ExternalInput").ap()
    zl = nc.dram_tensor("zl", [1, 128], BF16, kind="ExternalInput").ap()
    zr = nc.dram_tensor("zr", [1, 512], BF16, kind="ExternalInput").ap()
    onesrow = nc.dram_tensor("onesrow", [1, 128], BF16, kind="ExternalInput").ap()
    sel0 = nc.dram_tensor("sel0", [1, 128], F32, kind="ExternalInput").ap()
    sel1 = nc.dram_tensor("sel1", [1, 128], F32, kind="ExternalInput").ap()
    aconst = nc.dram_tensor("aconst", [1, HPC], F32, kind="ExternalInput").ap()
    out_d = nc.dram_tensor("out", [T, D_MODEL], F32, kind="ExternalOutput").ap()

    dbg = os.environ.get("KDEBUG", "") == "1"
    dbgt = {}
    if dbg:
        for nm, shp, dt_ in (
            ("d_qT", [128, T], F32),
            ("d_k1T", [128, T], F32),
            ("d_k2T", [128, T], F32),
            ("d_v", [128, T], BF16),
            ("d_p1", [128, (T // 128) * HPC * 512], BF16),
            ("d_p2", [128, (T // 128) * HPC * 512], BF16),
            ("d_outT", [128, T], F32),
            ("d_invd0", [1, T], F32),
            ("d_invd1", [1, T], F32),
            ("d_bc0", [128, 512], BF16),
            ("d_bc1", [128, 512], BF16),
        ):
            dbgt[nm] = nc.dram_tensor(nm, shp, dt_, kind="ExternalOutput").ap()

    with tile.TileContext(nc) as tc, ExitStack() as ctx:
        consts = ctx.enter_context(tc.tile_pool(name="consts", bufs=1))
        wq_s = consts.tile([128, 8, DC], F32R, tag="wq")
        wk1_s = consts.tile([128, 8, DC], F32R, tag="wk1")
        wk2_s = consts.tile([128, 8, DC], F32R, tag="wk2")
        wv_s = consts.tile([128, 8, DC], BF16, tag="wv")
        wo_s = consts.tile([128, D_MODEL], F32R, tag="wo")
        for dst, src in ((wq_s, wq), (wk1_s, wk1), (wk2_s, wk2), (wv_s, wv)):
            nc.sync.dma_start(out=dst, in_=src.rearrange("(c p) j -> p c j", p=128))
        nc.sync.dma_start(out=wo_s, in_=wo)
        lam_s = consts.tile([128, HPC], BF16, tag="lam")
        ones_s = consts.tile([128, 1], BF16, tag="ones")
        twos_s = consts.tile([128, 1], BF16, tag="twos")
        maskb_s = consts.tile([128, 128], BF16, tag="maskb")
        idneg_s = consts.tile([128, 128], BF16, tag="idneg")
        zl_s = consts.tile([1, 128], BF16, tag="zl")
        zr_s = consts.tile([1, 512], BF16, tag="zr")
        or_s = consts.tile([1, 128], BF16, tag="or")
        s0_s = consts.tile([1, 128], F32, tag="s0")
        s1c_s = consts.tile([1, 128], F32, tag="s1c")
        ac_s = consts.tile([1, HPC], F32, tag="ac")
        for dst, src in (
            (lam_s, lamones),
            (ones_s, ones1),
            (twos_s, twos),
            (maskb_s, maskb),
            (idneg_s, idneg),
            (zl_s, zl),
            (zr_s, zr),
            (or_s, onesrow),
            (s0_s, sel0),
            (s1c_s, sel1),
            (ac_s, aconst),
        ):
            nc.sync.dma_start(out=dst, in_=src)

        qk = ctx.enter_context(tc.tile_pool(name="qk", bufs=1))
        NCT = T // 512
        qTl = [qk.tile([128, 512], F32R, tag=f"qT{i}", name=f"qT{i}") for i in range(NCT)]
        k1Tl = [qk.tile([128, 512], F32R, tag=f"k1T{i}", name=f"k1T{i}") for i in range(NCT)]
        k2Tl = [qk.tile([128, 512], F32R, tag=f"k2T{i}", name=f"k2T{i}") for i in range(NCT)]
        vl = [qk.tile([128, 128], BF16, tag=f"v{i}", name=f"v{i}") for i in range(NKB)]

        # ---- Phase 1: projections (qT/k1T/k2T transposed [dims, T]; v [T, dims])
        with (
            tc.tile_pool(name="xt", bufs=1) as xtp,
            tc.tile_pool(name="pjp", bufs=4, space="PSUM") as pjp,
        ):
            xts = xtp.tile([128, 8, T], F32R, tag="xts")
            xtsb = xtp.tile([128, 8, T], BF16, tag="xtsb")
            for c in range(8):
                nc.sync.dma_start(out=xts[:, c, :], in_=xT[ts(c, 128), :])
                nc.sync.dma_start(out=xtsb[:, c, :], in_=xTb[ts(c, 128), :])

            alt = 0
            for ct in range(T // 512):
                for w_s, dstl in ((wk1_s, k1Tl), (wk2_s, k2Tl), (wq_s, qTl)):
                    ps = pjp.tile([128, 512], F32, tag="pj")
                    for c in range(8):
                        nc.tensor.matmul(
                            ps,
                            lhsT=w_s[:, c, :],
                            rhs=xts[:, c, ts(ct, 512)],
                            start=(c == 0),
                            stop=(c == 7),
                        )
                    if alt % 2 == 0:
                        nc.vector.tensor_copy(dstl[ct], ps)
                    else:
                        nc.scalar.copy(dstl[ct], ps)
                    alt += 1
                for tt in range(4 * ct, 4 * ct + 4):
                    psv = pjp.tile([128, 128], F32, tag="pjv")
                    for c in range(8):
                        nc.tensor.matmul(
                            psv,
                            lhsT=xtsb[:, c, ts(tt, 128)],
                            rhs=wv_s[:, c, :],
                            start=(c == 0),
                            stop=(c == 7),
                        )
                    if alt % 2 == 0:
                        nc.vector.tensor_copy(vl[tt], psv)
                    else:
                        nc.scalar.copy(vl[tt], psv)
                    alt += 1

        # ---- Phase 2: attention
        pp = ctx.enter_context(tc.tile_pool(name="pp", bufs=1))
        p1l = [pp.tile([128, HPC, 512], BF16, tag=f"p1_{k}", name=f"p1_{k}") for k in range(NKB)]
        p2l = [pp.tile([128, HPC, 512], BF16, tag=f"p2_{k}", name=f"p2_{k}") for k in range(NKB)]
        if dbg:
            for k in range(NKB):
                nc.vector.memset(p1l[k], 0.0)
                nc.vector.memset(p2l[k], 0.0)
        outTl = [pp.tile([128, 512], F32R, tag=f"outT{i}", name=f"outT{i}") for i in range(NQT)]
        invd_s = [
            [pp.tile([1, 512], F32, tag=f"invd{h}_{i}", name=f"invd{h}_{i}") for i in range(NQT)]
            for h in range(HPC)
        ]

        ctx2 = ExitStack()
        sp = ctx2.enter_context(tc.tile_pool(name="sp", bufs=1, space="PSUM"))
        accp = ctx2.enter_context(tc.tile_pool(name="accp", bufs=1, space="PSUM"))
        xpsp = ctx2.enter_context(tc.tile_pool(name="xpsp", bufs=1, space="PSUM"))
        rows = ctx2.enter_context(tc.tile_pool(name="rows", bufs=2))
        bcp = ctx2.enter_context(tc.tile_pool(name="bcp", bufs=2))
        fop = ctx2.enter_context(tc.tile_pool(name="fop", bufs=3))

        nqt_lim = int(os.environ.get("KQT", str(NQT)))
        for qt in range(min(NQT, nqt_lim)):
            nkb = (qt + 1) * (NQ // 128)  # k-blocks for this q-tile
            kb_d0 = qt * (NQ // 128)  # first diagonal k-block
            acc1 = accp.tile([128, 512], F32, tag="acc1")
            acc2 = accp.tile([128, 512], F32, tag="acc2")
            lmb = accp.tile([128, 512], F32, tag="lmb")
            # dummy zero matmuls: set has_written over whole banks so the
            # col-tiled accumulation streams below can all run start=False
            for bank in (acc1, acc2, lmb):
                nc.tensor.matmul(bank, lhsT=zl_s, rhs=zr_s, start=True, stop=False, skip_group_check=True)

            # pass 1: scores -> exp -> l/acc accumulation
            for kb in range(nkb):
                lo = max(0, 128 * (kb - kb_d0))
                s1 = sp.tile([128, 1024], F32, tag="s1")
                s2 = sp.tile([128, 1024], F32, tag="s2")
                for sb, kTl in ((s1, k1Tl), (s2, k2Tl)):
                    for h in range(HPC):
                        nc.tensor.matmul(
                            sb[:, ts(h, 512)],
                            lhsT=kTl[kb // 4][ts(h, 64), ts(kb % 4, 128)],
                            rhs=qTl[qt][ts(h, 64), :],
                            start=True,
                            stop=(kb < kb_d0),
                            tile_position=(h * 64, 0),
                        )
                        if kb >= kb_d0:
                            nc.tensor.matmul(
                                sb[:, h * 512 + lo : h * 512 + lo + 128],
                                lhsT=idneg_s,
                                rhs=maskb_s,
                                start=False,
                                stop=True,
                            )
                for sb, p_l in ((s1, p1l), (s2, p2l)):
                    if lo == 0:
                        nc.scalar.activation(
                            out=p_l[kb][:, :, :], in_=sb[:, :], func=AF.Exp, scale=SCALE
                        )
                    else:
                        for h in range(HPC):
                            nc.scalar.activation(
                                out=p_l[kb][:, h, lo:512],
                                in_=sb[:, h * 512 + lo : (h + 1) * 512],
                                func=AF.Exp,
                                scale=SCALE,
                            )
                last = kb == nkb - 1
                for h in range(HPC):
                    nc.tensor.matmul(
                        lmb[32 * h : 32 * h + 1, lo:512],
                        lhsT=lam_s[:, h : h + 1],
                        rhs=p1l[kb][:, h, lo:512],
                        start=False,
                        stop=last,
                        tile_position=(0, 32 * h),
                        skip_group_check=True,
                    )
                    nc.tensor.matmul(
                        lmb[64 + 32 * h : 64 + 32 * h + 1, lo:512],
                        lhsT=ones_s,
                        rhs=p2l[kb][:, h, lo:512],
                        start=False,
                        stop=last,
                        tile_position=(0, 64 + 32 * h),
                        skip_group_check=True,
                    )
                    nc.tensor.matmul(
                        acc1[ts(h, 64), lo:512],
                        lhsT=vl[kb][:, ts(h, 64)],
                        rhs=p1l[kb][:, h, lo:512],
                        start=False,
                        stop=last,
                        tile_position=(0, 64 * h),
                        skip_group_check=True,
                    )
                    nc.tensor.matmul(
                        acc2[ts(h, 64), lo:512],
                        lhsT=vl[kb][:, ts(h, 64)],
                        rhs=p2l[kb][:, h, lo:512],
                        start=False,
                        stop=last,
                        tile_position=(0, 64 * h),
                        skip_group_check=True,
                    )

            # row math: c = lam*l1/l2 (l1' rows at 0/32, l2 rows at 64/96)
            r2 = [rows.tile([1, 512], F32, tag=f"r2{h}", name=f"r2{h}") for h in range(HPC)]
            c_bf = [rows.tile([1, 512], BF16, tag=f"cbf{h}", name=f"cbf{h}") for h in range(HPC)]
            l1c = [rows.tile([1, 512], F32, tag=f"l1c{h}", name=f"l1c{h}") for h in range(HPC)]
            l2c = [rows.tile([1, 512], F32, tag=f"l2c{h}", name=f"l2c{h}") for h in range(HPC)]
            if os.environ.get("KFAKEC", "") == "1":
                for h in range(HPC):
                    nc.vector.memset(c_bf[h], 0.5)
                    nc.vector.memset(l1c[h], 1.0)
            else:
                for h in range(HPC):
                    l1r = lmb[32 * h : 32 * h + 1, :]
                    l2r = lmb[64 + 32 * h : 64 + 32 * h + 1, :]
                    nc.scalar.copy(l2c[h], l2r)
                    nc.vector.reciprocal_approx_fast(out=r2[h], in_=l2c[h])
                    nc.vector.tensor_tensor(c_bf[h], l1r, r2[h], op=ALU.mult)
                    nc.scalar.copy(l1c[h], l1r)

            # broadcast c along partitions via K=1 matmul (no DRAM roundtrip)
            bc = []
            for h in range(HPC):
                bps = xpsp.tile([128, 512], F32, tag="xps", name=f"bps{h}")
                nc.tensor.matmul(bps, lhsT=or_s, rhs=c_bf[h], start=True, stop=True)
                bch = bcp.tile([128, 512], BF16, tag=f"bc{h}", name=f"bc{h}")
                nc.vector.tensor_copy(bch, bps)
                bc.append(bch)
            if dbg and qt == 0:
                nc.sync.dma_start(out=dbgt["d_bc0"], in_=bc[0])
                nc.sync.dma_start(out=dbgt["d_bc1"], in_=bc[1])

            # out numerator: outT = acc1 - c*acc2  (per-head c broadcast)
            skip_outc = os.environ.get("KNOOUTC", "") == "1"
            tmp = rows.tile([128, 512], F32, tag="octmp")
            if not skip_outc:
                for h in range(HPC):
                    nc.vector.tensor_tensor(
                        tmp[ts(h, 64), :], acc2[ts(h, 64), :], bc[h][0:64, :], op=ALU.mult
                    )
                nc.vector.tensor_tensor(
                    outTl[qt], acc1[:, :], tmp[:, :], op=ALU.subtract
                )

            # msums accumulate in an xps-bank tile (lmb frees early for qt+1)
            msq = xpsp.tile([128, 512], F32, tag="xps", name="msq")
            nc.tensor.matmul(msq, lhsT=zl_s, rhs=zr_s, start=True, stop=False, skip_group_check=True)

            # pass 2: t = c*p2 (in place), m = min(p1, t) (in place), msum.
            # Per-kb tiles make the qt+1-overwrite deps precise.
            for kb in range(nkb):
                lo = max(0, 128 * (kb - kb_d0))
                if os.environ.get("KNOPASS2", "") != "1":
                    for h in range(HPC):
                        blk = p2l[kb][:, h, lo:512]
                        nc.vector.tensor_tensor(
                            blk, blk, bc[h][:, lo:512], op=ALU.mult
                        )
                        nc.vector.tensor_tensor(
                            blk, p1l[kb][:, h, lo:512], blk, op=ALU.min
                        )
                for h in range(HPC):
                    nc.tensor.matmul(
                        msq[32 * h : 32 * h + 1, lo:512],
                        lhsT=twos_s,
                        rhs=p2l[kb][:, h, lo:512],
                        start=False,
                        stop=(kb == nkb - 1),
                        tile_position=(0, 32 * h),
                        skip_group_check=True,
                    )

            # denom = (1+lam)*l1 - 2*sum(min) ; invd = 1/denom
            dn = [rows.tile([1, 512], F32, tag=f"dn{h}", name=f"dn{h}") for h in range(HPC)]
            for h in range(HPC):
                nc.vector.scalar_tensor_tensor(
                    out=dn[h],
                    in0=l1c[h],
                    scalar=ac_s[0:1, h : h + 1],
                    in1=msq[32 * h : 32 * h + 1, :],
                    op0=ALU.mult,
                    op1=ALU.subtract,
                )
                nc.vector.reciprocal_approx_fast(out=invd_s[h][qt], in_=dn[h])

            # per-qt tail: bcinv chunk, scale outT cols, output projection
            bciq = xpsp.tile([128, 512], F32, tag="xps", name="bciq")
            nc.tensor.matmul(
                bciq, lhsT=s0_s, rhs=invd_s[0][qt], start=True, stop=False
            )
            nc.tensor.matmul(
                bciq, lhsT=s1c_s, rhs=invd_s[1][qt], start=False, stop=True
            )
            bci_sb = rows.tile([128, 512], F32, tag="bcisb")
            nc.scalar.copy(bci_sb, bciq)
            oq = outTl[qt]
            nc.vector.tensor_tensor(oq, oq, bci_sb, op=ALU.mult)
            if os.environ.get("KNOFINAL", "") == "1":
                continue
            for tci in range(NQ // 128):
                tt = qt * (NQ // 128) + tci
                fstage = fop.tile([128, D_MODEL], F32, tag="fstage")
                for jh in range(D_MODEL // 512):
                    fp = xpsp.tile([128, 512], F32, tag="xps", name="fp")
                    nc.tensor.matmul(
                        fp,
                        lhsT=outTl[qt][:, ts(tci, 128)],
                        rhs=wo_s[:, ts(jh, 512)],
                        start=True,
                        stop=True,
                    )
                    if (tt + jh) % 2 == 0:
                        nc.vector.tensor_copy(fstage[:, ts(jh, 512)], fp)
                    else:
                        nc.scalar.copy(fstage[:, ts(jh, 512)], fp)
                nc.sync.dma_start(out=out_d[ts(tt, 128), :], in_=fstage)

        if dbg:
            for i in range(NCT):
                nc.sync.dma_start(out=dbgt["d_qT"][:, ts(i, 512)], in_=qTl[i].bitcast(F32))
                nc.sync.dma_start(out=dbgt["d_k1T"][:, ts(i, 512)], in_=k1Tl[i].bitcast(F32))
                nc.sync.dma_start(out=dbgt["d_k2T"][:, ts(i, 512)], in_=k2Tl[i].bitcast(F32))
            for k in range(NKB):
                nc.sync.dma_start(out=dbgt["d_v"][:, ts(k, 128)], in_=vl[k])
                nc.sync.dma_start(out=dbgt["d_p1"][:, ts(k, HPC * 512)], in_=p1l[k].rearrange("p a b -> p (a b)"))
                nc.sync.dma_start(out=dbgt["d_p2"][:, ts(k, HPC * 512)], in_=p2l[k].rearrange("p a b -> p (a b)"))
            for i in range(NQT):
                nc.sync.dma_start(out=dbgt["d_outT"][:, ts(i, 512)], in_=outTl[i].bitcast(F32))
                nc.sync.dma_start(out=dbgt["d_invd0"][0:1, ts(i, 512)], in_=invd_s[0][i])
                nc.sync.dma_start(out=dbgt["d_invd1"][0:1, ts(i, 512)], in_=invd_s[1][i])

        ctx2.close()

    nc.compile()
    return nc


_CACHE: dict = {}


def _get_nc(T: int):
    if T not in _CACHE:
        _CACHE[T] = _build(T)
    return _CACHE[T]


def make_in_maps(x, Wq, Wk1, Wk2, Wv, Wo, lambda_logit):
    x = np.asarray(x, np.float32)
    B, T, D = x.shape
    assert B == 1 and D == D_MODEL
    Wq = np.asarray(Wq, np.float32)
    Wk1 = np.asarray(Wk1, np.float32)
    Wk2 = np.asarray(Wk2, np.float32)
    Wv = np.asarray(Wv, np.float32)
    Wo = np.asarray(Wo, np.float32)
    lam = 1.0 / (1.0 + np.exp(-np.asarray(lambda_logit, np.float64)))

    xT = np.ascontiguousarray(x[0].T)
    xTb = xT.astype(BF16NP)
    maskb_np = np.tril(np.ones((128, 128), np.float32), -1).astype(BF16NP)
    idneg_np = (NEG * np.eye(128, dtype=np.float32)).astype(BF16NP)

    in_maps = []
    for c in range(N_CORES):
        sl = slice(DC * c, DC * (c + 1))
        lam2 = lam[HPC * c : HPC * (c + 1)]
        in_maps.append(
            {
                "xT": xT,
                "xTb": xTb,
                "wq": np.ascontiguousarray(Wq[sl].T),
                "wk1": np.ascontiguousarray(Wk1[sl].T),
                "wk2": np.ascontiguousarray(Wk2[sl].T),
                "wv": np.ascontiguousarray(Wv[sl].T).astype(BF16NP),
                "wo": np.ascontiguousarray(Wo[:, sl].T),
                "lamones": np.tile(
                    lam2[None, :].astype(np.float32), (128, 1)
                ).astype(BF16NP),
                "ones1": np.ones((128, 1), BF16NP),
                "twos": np.full((128, 1), 2.0, BF16NP),
                "maskb": maskb_np,
                "idneg": idneg_np,
                "zl": np.zeros((1, 128), BF16NP),
                "zr": np.zeros((1, 512), BF16NP),
                "onesrow": np.ones((1, 128), BF16NP),
                "sel0": np.concatenate(
                    [np.ones(64, np.float32), np.zeros(64, np.float32)]
                ).reshape(1, 128),
                "sel1": np.concatenate(
                    [np.zeros(64, np.float32), np.ones(64, np.float32)]
                ).reshape(1, 128),
                "aconst": ((1.0 + lam2) / lam2).astype(np.float32).reshape(1, HPC),
            }
        )
    return in_maps, T


def kernel(x, Wq, Wk1, Wk2, Wv, Wo, lambda_logit):
    in_maps, T = make_in_maps(x, Wq, Wk1, Wk2, Wv, Wo, lambda_logit)
    nc = _get_nc(T)
    last_err = None
    for _attempt in range(3):
        try:
            res = run_bass_kernel_spmd(nc, in_maps, list(range(N_CORES))).results
            break
        except Exception as e:  # transient NRT/axon wedges; retry
            last_err = e
    else:
        raise last_err
    out = np.zeros((T, D_MODEL), np.float32)
    for r in res:
        out += np.asarray(r["out"], np.float32)
    return out.reshape(1, T, D_MODEL)

